# revision 18
# baseline (speedup 1.0000x reference)
"""Trainium2 Bass kernel for nn_Decoder_14894946583396 (dense_mlp).

Reference computation:
    sized = broadcast(representation[B,1,R] -> [B,S,R])   (ones @ rep)
    h     = relu(sized @ W1^T + b1)                       [B,S,HID]
    out   = h @ W2^T + b2                                 [B,S,OUT]

Every position s within batch b receives the identical input row, so
    row[b] = relu(rep[b] @ W1^T + b1) @ W2^T + b2         [B,OUT]
    out[b, s, :] = row[b]  for all s

Data-parallel across 8 NeuronCores: 4 batches per core, replicated
weights.  The 8 MiB bf16 output stream is the roofline; the schedule
minimizes head latency before the first output DMA and keeps the DMA
descriptors >= 4 KiB (small descriptors measurably cut per-engine DMA
throughput from ~25 GB/s to ~18 GB/s):

  * Inputs ride the sync HWDGE ring (xt, aux, W1 in 2 halves) --
    the highest-priority queue, drained first -- and w2's two halves
    ride the scalar HWDGE ring.  DMA queues drain in strict priority
    order (sync > scalar > SWDGE), and a DMA's completion semaphore
    fires ~1.8us after its last byte (HBM receipt round trip), so the
    weight layout is chosen to put W1's last byte as early as
    possible.
  * L1 is rc-major over 2 chunks; b1 rides mid-accumulation-group as
    a K=1 ones-matmul; relu on ACT casts to bf16.
  * L2 produces the REPLICATED output directly: for each (batch, oc)
    a 4-matmul group uses lhsT = ht[:, column b] broadcast along the
    free dim to 128 columns, so PSUM receives row[b] already
    replicated across all 128 partitions.  No selector matmul, no
    intermediate y tile, two fewer engine hops before the first
    output byte.  b2 opens each group as a K=1 start-matmul (needs
    only aux, so it runs before relu even lands).
  * One DVE copy per (batch, oc) writes the PSUM block TWICE into a
    double-row tile yt2[b] = [row | row], giving the output DMA a
    real 2048-elem contiguous source run -> 4 KiB descriptors.
  * 4 output DMAs (one per batch, 2 MiB each): b0 on sync, b1 on
    scalar (HWDGE count = 6 inputs + 2 outputs = exactly the 8
    lanes), b2/b3 on the SWDGE ring.  Each carries only its copy-tick
    wait.
  * Output stored bf16 (halves the write stream); host upcasts during
    the gather.  Total rounding ~4e-3, well under the 2e-2 gate.

A chain of 1-wait SP nops pre-observes every final tick for the
TileContext exit drain.
"""

import sys

import numpy as np

if "/opt/trn_rl_repo" not in sys.path:
    sys.path.insert(0, "/opt/trn_rl_repo")

B, S, R = 32, 1024, 1024
HID, OUT = 512, 1024
N_CORES = 8
BPC = B // N_CORES  # batches per core

RC = R // 128  # layer-1 contraction chunks
HC = HID // 128  # layer-2 contraction chunks
OC = OUT // 512  # 512-wide output column chunks

N_COPIES = S // 128  # broadcast factor per output DMA (0-stride AP)

ONE0 = RC * BPC  # ones row offset in xt
XT_W = ONE0 + BPC  # x^T | ones

# aux row 0: b1 (cols 0-511) | b2 (512-1535) | ones128 (1536-1663)
AUX_ONES = HID + OUT
AUX_W = HID + OUT + 128

_CACHED_NC = None


def _build_nc():
    import concourse.bass as bass
    import concourse.mybir as mybir
    from concourse.tile import TileContext, add_dep_helper

    f32 = mybir.dt.float32
    bf16 = mybir.dt.bfloat16
    relu = mybir.ActivationFunctionType.Relu
    nc = bass.Bass()

    aux = nc.dram_tensor("aux", [1, AUX_W], bf16, kind="ExternalInput")
    # xt (x^T | ones) rides as the head columns of the w1 tensor -- one
    # DMA, one completion lane; L1 needs both at the same time anyway
    w1 = nc.dram_tensor("w1", [128, XT_W + RC * HID], bf16, kind="ExternalInput")
    w2 = nc.dram_tensor("w2", [128, HC * OUT], bf16, kind="ExternalInput")
    out = nc.dram_tensor("out", [BPC, S, OUT], bf16, kind="ExternalOutput")

    with TileContext(nc) as tc:
        with (
            tc.tile_pool(name="const", bufs=1) as cpool,
            tc.tile_pool(name="psum_h", bufs=1, space="PSUM") as pp_h,
            tc.tile_pool(name="psum_bc", bufs=3, space="PSUM") as pp_bc,
        ):
            # ---- input DMAs ------------------------------------------------
            # EVERYTHING rides the sync ring as one deep FIFO: aux (tiny,
            # first), xt+W1 (one DMA), W2, then the five output DMAs.
            # Queues drain serially in long bursts, so a single deep queue
            # keeps all 16 SDMA engines ~99% busy with no handoff gaps,
            # and the FIFO order IS the dependency order.
            aux_sb = cpool.tile([1, AUX_W], bf16, tag="aux")
            dma_aux = nc.sync.dma_start(out=aux_sb[0:1, :], in_=aux[0:1, :])
            w1_sb = cpool.tile([128, XT_W + RC * HID], bf16, tag="w1")
            w1_mid = XT_W + (RC // 2) * HID
            w1_dmas = [
                nc.sync.dma_start(out=w1_sb[:, 0:w1_mid], in_=w1[:, 0:w1_mid]),
                nc.sync.dma_start(out=w1_sb[:, w1_mid:], in_=w1[:, w1_mid:]),
            ]
            xt_sb = w1_sb  # xt = head columns of the w1 tile
            w2_sb = cpool.tile([128, HC * OUT], bf16, tag="w2")
            w2_cols = HC * OUT // OC
            w2_dmas = [
                nc.sync.dma_start(
                    out=w2_sb[:, c * w2_cols : (c + 1) * w2_cols],
                    in_=w2[:, c * w2_cols : (c + 1) * w2_cols],
                )
                for c in range(OC)
            ]
            dma_xt = w1_dmas[0]

            # ---- L1: H^T[h, m] = W1 @ x + b1, relu -------------------------
            ph = [
                pp_h.tile([128, BPC], f32, tag=f"h{hc}", name=f"ph{hc}")
                for hc in range(HC)
            ]
            # rc-major; one PSUM
            # bank per hc keeps each accumulation group sequential within
            # its bank.
            for rc in range(RC - 1):
                for hc in range(HC):
                    nc.tensor.matmul(
                        ph[hc][:, :],
                        lhsT=w1_sb[:, XT_W + rc * HID + hc * 128 : XT_W + rc * HID + (hc + 1) * 128],
                        rhs=xt_sb[:, rc * BPC : (rc + 1) * BPC],
                        start=(rc == 0),
                        stop=False,
                        skip_group_check=True,
                    )
            # b1 rides mid-group: ph[h, m] += b1[h] * 1
            for hc in range(HC):
                nc.tensor.matmul(
                    ph[hc][:, :],
                    lhsT=aux_sb[0:1, hc * 128 : (hc + 1) * 128],
                    rhs=xt_sb[0:1, ONE0 : ONE0 + BPC],
                    start=False,
                    stop=False,
                    skip_group_check=True,
                )
            rc = RC - 1
            for hc in range(HC):
                nc.tensor.matmul(
                    ph[hc][:, :],
                    lhsT=w1_sb[:, XT_W + rc * HID + hc * 128 : XT_W + rc * HID + (hc + 1) * 128],
                    rhs=xt_sb[:, rc * BPC : (rc + 1) * BPC],
                    start=False,
                    stop=True,
                    skip_group_check=True,
                )
            ht_sb = cpool.tile([128, HC * BPC], bf16, tag="ht")
            relus = []
            for hc in range(HC):
                r = nc.scalar.activation(
                    ht_sb[:, hc * BPC : (hc + 1) * BPC],
                    ph[hc][:, :],
                    relu,
                )
                relus.append(r)

            # ---- L2, replicated directly, per (batch, oc half) -------------
            yts = [
                cpool.tile([128, OUT], bf16, tag=f"yt{b}", name=f"yt{b}")
                for b in range(BPC)
            ]
            out_dmas = []
            last_dve = None
            w2_seen = [False] * OC
            for b in range(BPC):
                for oc in range(OC):
                    pb = pp_bc.tile([128, 512], f32, tag="bc", name=f"pb{b}_{oc}")
                    # b2 opens the group (needs only aux -- runs early)
                    nc.tensor.matmul(
                        pb[:, :],
                        lhsT=aux_sb[0:1, AUX_ONES : AUX_ONES + 128],
                        rhs=aux_sb[0:1, HID + oc * 512 : HID + (oc + 1) * 512],
                        start=True,
                        stop=False,
                    )
                    if not w2_seen[oc]:
                        # PE stalls here only on the w2 half it is about to
                        # consume; keeps the matmuls' wait slot free for
                        # their relu-tick wait.
                        w2_seen[oc] = True
                        wn2 = nc.tensor.nop(nofuse=True)
                        add_dep_helper(
                            wn2.ins,
                            w2_dmas[oc].ins,
                            sync=True,
                            reason=f"observe w2-{oc}",
                        )
                    for hc in range(HC):
                        # lhsT = ht column for batch b, broadcast along the
                        # free dim -> PSUM gets row[b] replicated across all
                        # 128 partitions.  No selector matmul needed.
                        nc.tensor.matmul(
                            pb[:, :],
                            lhsT=ht_sb[
                                :, hc * BPC + b : hc * BPC + b + 1
                            ].to_broadcast((128, 128)),
                            rhs=w2_sb[
                                :,
                                oc * HC * 512
                                + hc * 512 : oc * HC * 512
                                + (hc + 1) * 512,
                            ],
                            start=False,
                            stop=(hc == HC - 1),
                        )
                    # plain full-tile read: the PSUM dep tracker sees the
                    # copy as covering the writer, so the next group's
                    # start-matmul carries only this copy-tick wait
                    last_dve = nc.vector.tensor_copy(
                        yts[b][:, oc * 512 : (oc + 1) * 512], pb[:, :]
                    )
                    if b == 0:
                        # batch 0 ships per oc half (1 MiB) the moment its
                        # copy lands -- trims ~2us off the first output's
                        # start at a small descriptor-size cost
                        d = nc.sync.dma_start(
                            out=out[0, :, oc * 512 : (oc + 1) * 512].rearrange(
                                "(p c) o -> p c o", c=N_COPIES
                            ),
                            in_=yts[0][:, oc * 512 : (oc + 1) * 512]
                            .rearrange("p (c o) -> p c o", c=1)
                            .to_broadcast((128, N_COPIES, 512)),
                        )
                        out_dmas.append(d)
                if b == 0:
                    continue
                # one 2 MiB DMA per later batch writes all S rows via a
                # 0-stride broadcast source AP (2 KiB contiguous runs on
                # the DRAM side); both oc copies are DVE so the DMA
                # carries one wait
                d = (nc.gpsimd if b >= BPC - 2 else nc.sync).dma_start(
                    out=out[b, :, :].rearrange("(p c) o -> p c o", c=N_COPIES),
                    in_=yts[b][:, :]
                    .rearrange("p (c o) -> p c o", c=1)
                    .to_broadcast((128, N_COPIES, OUT)),
                )
                out_dmas.append(d)

            # single-sync-wait discipline for the TileContext exit drain:
            # chain SP nops, one dependency each, so SP's vector clock
            # observes every DMA lane / engine tick before the drain.
            tail = (
                out_dmas
                + w1_dmas
                + w2_dmas
                + [dma_xt, dma_aux, relus[-1], last_dve]
            )
            tail = [t for t in tail if t is not None]
            for d in tail:
                tn = nc.sync.nop(nofuse=True)
                add_dep_helper(
                    tn.ins, d.ins, sync=True, reason="observe final ticks pre-drain"
                )

    return nc


def _get_nc():
    global _CACHED_NC
    if _CACHED_NC is None:
        _CACHED_NC = _build_nc()
    return _CACHED_NC


def _prep_in_maps(representation, W1, b1, W2, b2):
    import ml_dtypes

    bf16 = ml_dtypes.bfloat16

    rep = np.asarray(representation, dtype=np.float32).reshape(B, R)
    w1 = np.asarray(W1, dtype=np.float32)
    w2 = np.asarray(W2, dtype=np.float32)
    b1 = np.asarray(b1, dtype=np.float32)
    b2 = np.asarray(b2, dtype=np.float32)

    # w1p[p, rc*HID + hc*128 + j] = W1[hc*128+j, rc*128+p]
    w1p = np.ascontiguousarray(
        w1.reshape(HC, 128, RC, 128).transpose(3, 2, 0, 1).reshape(128, RC * HID)
    ).astype(bf16)
    w1p = np.concatenate(
        [np.zeros((128, XT_W), dtype=bf16), w1p], axis=1
    )  # head columns hold xt, filled per core
    # w2p[p, oc*HC*512 + hc*512 + o] = W2[oc*512+o, hc*128+p]  (oc-major)
    w2p = np.ascontiguousarray(
        w2.reshape(OC, 512, HC, 128).transpose(3, 0, 2, 1).reshape(128, HC * OUT)
    ).astype(bf16)
    # aux row: b1 | b2 | ones128
    auxp = np.zeros((1, AUX_W), dtype=np.float32)
    auxp[0, 0:HID] = b1
    auxp[0, HID : HID + OUT] = b2
    auxp[0, AUX_ONES : AUX_ONES + 128] = 1.0

    in_maps = []
    for c in range(N_CORES):
        xtc = rep[c * BPC : (c + 1) * BPC].T  # [R, BPC]
        # xt[p, rc*BPC + m] = rep[m, rc*128+p] | ones row
        xtp = np.zeros((128, XT_W), dtype=np.float32)
        xtp[:, 0 : RC * BPC] = (
            xtc.reshape(RC, 128, BPC).transpose(1, 0, 2).reshape(128, RC * BPC)
        )
        xtp[0, ONE0 : ONE0 + BPC] = 1.0
        w1c = w1p.copy()
        w1c[:, 0:XT_W] = xtp.astype(bf16)
        in_maps.append(
            {
                "aux": auxp.astype(bf16),
                "w1": w1c,
                "w2": w2p,
            }
        )
    return in_maps


def run_sharded(representation, W1, b1, W2, b2, **run_kwargs):
    """Compile+run on 8 cores; returns (full_output, BassKernelResults)."""
    from concourse.bass_utils import run_bass_kernel_spmd

    nc = _get_nc()
    in_maps = _prep_in_maps(representation, W1, b1, W2, b2)
    res = run_bass_kernel_spmd(nc, in_maps, core_ids=list(range(N_CORES)), **run_kwargs)
    full = np.concatenate(
        [np.asarray(r["out"]).astype(np.float32) for r in res.results], axis=0
    )
    return full, res


def kernel(representation, size_matrix=None, W1=None, b1=None, W2=None, b2=None):
    # size_matrix only contributes its shape in the reference (ones_like);
    # its values are unused.
    full, _ = run_sharded(representation, W1, b1, W2, b2)
    return full


# revision 19
# speedup vs baseline: 1.0173x; 1.0173x over previous
"""Trainium2 Bass kernel for nn_Decoder_14894946583396 (dense_mlp).

Reference computation:
    sized = broadcast(representation[B,1,R] -> [B,S,R])   (ones @ rep)
    h     = relu(sized @ W1^T + b1)                       [B,S,HID]
    out   = h @ W2^T + b2                                 [B,S,OUT]

Every position s within batch b receives the identical input row, so
    row[b] = relu(rep[b] @ W1^T + b1) @ W2^T + b2         [B,OUT]
    out[b, s, :] = row[b]  for all s

Data-parallel across 8 NeuronCores: 4 batches per core, replicated
weights.  The 8 MiB bf16 output stream is the roofline; the schedule
minimizes head latency before the first output DMA and keeps the DMA
descriptors >= 4 KiB (small descriptors measurably cut per-engine DMA
throughput from ~25 GB/s to ~18 GB/s):

  * Inputs ride the sync HWDGE ring (xt, aux, W1 in 2 halves) --
    the highest-priority queue, drained first -- and w2's two halves
    ride the scalar HWDGE ring.  DMA queues drain in strict priority
    order (sync > scalar > SWDGE), and a DMA's completion semaphore
    fires ~1.8us after its last byte (HBM receipt round trip), so the
    weight layout is chosen to put W1's last byte as early as
    possible.
  * L1 is rc-major over 2 chunks; b1 rides mid-accumulation-group as
    a K=1 ones-matmul; relu on ACT casts to bf16.
  * L2 produces the REPLICATED output directly: for each (batch, oc)
    a 4-matmul group uses lhsT = ht[:, column b] broadcast along the
    free dim to 128 columns, so PSUM receives row[b] already
    replicated across all 128 partitions.  No selector matmul, no
    intermediate y tile, two fewer engine hops before the first
    output byte.  b2 opens each group as a K=1 start-matmul (needs
    only aux, so it runs before relu even lands).
  * One DVE copy per (batch, oc) writes the PSUM block TWICE into a
    double-row tile yt2[b] = [row | row], giving the output DMA a
    real 2048-elem contiguous source run -> 4 KiB descriptors.
  * 4 output DMAs (one per batch, 2 MiB each): b0 on sync, b1 on
    scalar (HWDGE count = 6 inputs + 2 outputs = exactly the 8
    lanes), b2/b3 on the SWDGE ring.  Each carries only its copy-tick
    wait.
  * Output stored bf16 (halves the write stream); host upcasts during
    the gather.  Total rounding ~4e-3, well under the 2e-2 gate.

A chain of 1-wait SP nops pre-observes every final tick for the
TileContext exit drain.
"""

import sys

import numpy as np

if "/opt/trn_rl_repo" not in sys.path:
    sys.path.insert(0, "/opt/trn_rl_repo")

B, S, R = 32, 1024, 1024
HID, OUT = 512, 1024
N_CORES = 8
BPC = B // N_CORES  # batches per core

RC = R // 128  # layer-1 contraction chunks
HC = HID // 128  # layer-2 contraction chunks
OC = OUT // 512  # 512-wide output column chunks

N_COPIES = S // 128  # broadcast factor per output DMA (0-stride AP)

ONE0 = RC * BPC  # ones row offset in xt
XT_W = ONE0 + BPC  # x^T | ones

# aux row 0: b1 (cols 0-511) | b2 (512-1535) | ones128 (1536-1663)
AUX_ONES = HID + OUT
AUX_W = HID + OUT + 128

_CACHED_NC = None


def _build_nc():
    import concourse.bass as bass
    import concourse.mybir as mybir
    from concourse.tile import TileContext, add_dep_helper

    f32 = mybir.dt.float32
    bf16 = mybir.dt.bfloat16
    relu = mybir.ActivationFunctionType.Relu
    nc = bass.Bass()

    aux = nc.dram_tensor("aux", [1, AUX_W], bf16, kind="ExternalInput")
    # xt (x^T | ones) rides as the head columns of the w1 tensor -- one
    # DMA, one completion lane; L1 needs both at the same time anyway
    w1 = nc.dram_tensor("w1", [128, XT_W + RC * HID], bf16, kind="ExternalInput")
    w2 = nc.dram_tensor("w2", [128, HC * OUT], bf16, kind="ExternalInput")
    out = nc.dram_tensor("out", [BPC, S, OUT], bf16, kind="ExternalOutput")

    with TileContext(nc) as tc:
        with (
            tc.tile_pool(name="const", bufs=1) as cpool,
            tc.tile_pool(name="psum_h", bufs=1, space="PSUM") as pp_h,
            tc.tile_pool(name="psum_bc", bufs=3, space="PSUM") as pp_bc,
        ):
            # ---- input DMAs ------------------------------------------------
            # EVERYTHING rides the sync ring as one deep FIFO: aux (tiny,
            # first), xt+W1 (one DMA), W2, then the five output DMAs.
            # Queues drain serially in long bursts, so a single deep queue
            # keeps all 16 SDMA engines ~99% busy with no handoff gaps,
            # and the FIFO order IS the dependency order.
            aux_sb = cpool.tile([1, AUX_W], bf16, tag="aux")
            dma_aux = nc.sync.dma_start(out=aux_sb[0:1, :], in_=aux[0:1, :])
            w1_sb = cpool.tile([128, XT_W + RC * HID], bf16, tag="w1")
            w1_mid = XT_W + (RC // 2) * HID
            w1_dmas = [
                nc.sync.dma_start(out=w1_sb[:, 0:w1_mid], in_=w1[:, 0:w1_mid]),
                nc.sync.dma_start(out=w1_sb[:, w1_mid:], in_=w1[:, w1_mid:]),
            ]
            xt_sb = w1_sb  # xt = head columns of the w1 tile
            w2_sb = cpool.tile([128, HC * OUT], bf16, tag="w2")
            w2_dmas = [nc.sync.dma_start(out=w2_sb[:, :], in_=w2[:, :])]
            dma_xt = w1_dmas[0]

            # ---- L1: H^T[h, m] = W1 @ x + b1, relu -------------------------
            ph = [
                pp_h.tile([128, BPC], f32, tag=f"h{hc}", name=f"ph{hc}")
                for hc in range(HC)
            ]
            # rc-major; one PSUM
            # bank per hc keeps each accumulation group sequential within
            # its bank.
            for rc in range(RC - 1):
                for hc in range(HC):
                    nc.tensor.matmul(
                        ph[hc][:, :],
                        lhsT=w1_sb[:, XT_W + rc * HID + hc * 128 : XT_W + rc * HID + (hc + 1) * 128],
                        rhs=xt_sb[:, rc * BPC : (rc + 1) * BPC],
                        start=(rc == 0),
                        stop=False,
                        skip_group_check=True,
                    )
            # b1 rides mid-group: ph[h, m] += b1[h] * 1
            for hc in range(HC):
                nc.tensor.matmul(
                    ph[hc][:, :],
                    lhsT=aux_sb[0:1, hc * 128 : (hc + 1) * 128],
                    rhs=xt_sb[0:1, ONE0 : ONE0 + BPC],
                    start=False,
                    stop=False,
                    skip_group_check=True,
                )
            rc = RC - 1
            for hc in range(HC):
                nc.tensor.matmul(
                    ph[hc][:, :],
                    lhsT=w1_sb[:, XT_W + rc * HID + hc * 128 : XT_W + rc * HID + (hc + 1) * 128],
                    rhs=xt_sb[:, rc * BPC : (rc + 1) * BPC],
                    start=False,
                    stop=True,
                    skip_group_check=True,
                )
            ht_sb = cpool.tile([128, HC * BPC], bf16, tag="ht")
            relus = []
            for hc in range(HC):
                r = nc.scalar.activation(
                    ht_sb[:, hc * BPC : (hc + 1) * BPC],
                    ph[hc][:, :],
                    relu,
                )
                relus.append(r)

            # ---- L2, replicated directly, per (batch, oc half) -------------
            yts = [
                cpool.tile([128, OUT], bf16, tag=f"yt{b}", name=f"yt{b}")
                for b in range(BPC)
            ]
            out_dmas = []
            last_dve = None
            w2_seen = [False] * OC
            for b in range(BPC):
                for oc in range(OC):
                    pb = pp_bc.tile([128, 512], f32, tag="bc", name=f"pb{b}_{oc}")
                    # b2 opens the group (needs only aux -- runs early)
                    nc.tensor.matmul(
                        pb[:, :],
                        lhsT=aux_sb[0:1, AUX_ONES : AUX_ONES + 128],
                        rhs=aux_sb[0:1, HID + oc * 512 : HID + (oc + 1) * 512],
                        start=True,
                        stop=False,
                    )
                    if not w2_seen[oc]:
                        # PE stalls here only on the w2 half it is about to
                        # consume; keeps the matmuls' wait slot free for
                        # their relu-tick wait.
                        w2_seen[oc] = True
                        wn2 = nc.tensor.nop(nofuse=True)
                        add_dep_helper(
                            wn2.ins,
                            w2_dmas[0].ins,
                            sync=True,
                            reason=f"observe w2-{oc}",
                        )
                    for hc in range(HC):
                        # lhsT = ht column for batch b, broadcast along the
                        # free dim -> PSUM gets row[b] replicated across all
                        # 128 partitions.  No selector matmul needed.
                        nc.tensor.matmul(
                            pb[:, :],
                            lhsT=ht_sb[
                                :, hc * BPC + b : hc * BPC + b + 1
                            ].to_broadcast((128, 128)),
                            rhs=w2_sb[
                                :,
                                oc * HC * 512
                                + hc * 512 : oc * HC * 512
                                + (hc + 1) * 512,
                            ],
                            start=False,
                            stop=(hc == HC - 1),
                        )
                    # plain full-tile read: the PSUM dep tracker sees the
                    # copy as covering the writer, so the next group's
                    # start-matmul carries only this copy-tick wait
                    last_dve = nc.vector.tensor_copy(
                        yts[b][:, oc * 512 : (oc + 1) * 512], pb[:, :]
                    )
                    if b == 0:
                        # batch 0 ships per oc half (1 MiB) the moment its
                        # copy lands -- trims ~2us off the first output's
                        # start at a small descriptor-size cost
                        d = nc.sync.dma_start(
                            out=out[0, :, oc * 512 : (oc + 1) * 512].rearrange(
                                "(p c) o -> p c o", c=N_COPIES
                            ),
                            in_=yts[0][:, oc * 512 : (oc + 1) * 512]
                            .rearrange("p (c o) -> p c o", c=1)
                            .to_broadcast((128, N_COPIES, 512)),
                        )
                        out_dmas.append(d)
                if b == 0:
                    continue
                # one 2 MiB DMA per later batch writes all S rows via a
                # 0-stride broadcast source AP (2 KiB contiguous runs on
                # the DRAM side); both oc copies are DVE so the DMA
                # carries one wait
                d = (nc.gpsimd if b == BPC - 1 else nc.sync).dma_start(
                    out=out[b, :, :].rearrange("(p c) o -> p c o", c=N_COPIES),
                    in_=yts[b][:, :]
                    .rearrange("p (c o) -> p c o", c=1)
                    .to_broadcast((128, N_COPIES, OUT)),
                )
                out_dmas.append(d)

            # single-sync-wait discipline for the TileContext exit drain:
            # chain SP nops, one dependency each, so SP's vector clock
            # observes every DMA lane / engine tick before the drain.
            tail = (
                out_dmas
                + w1_dmas
                + w2_dmas
                + [dma_xt, dma_aux, relus[-1], last_dve]
            )
            tail = [t for t in tail if t is not None]
            for d in tail:
                tn = nc.sync.nop(nofuse=True)
                add_dep_helper(
                    tn.ins, d.ins, sync=True, reason="observe final ticks pre-drain"
                )

    return nc


def _get_nc():
    global _CACHED_NC
    if _CACHED_NC is None:
        _CACHED_NC = _build_nc()
    return _CACHED_NC


def _prep_in_maps(representation, W1, b1, W2, b2):
    import ml_dtypes

    bf16 = ml_dtypes.bfloat16

    rep = np.asarray(representation, dtype=np.float32).reshape(B, R)
    w1 = np.asarray(W1, dtype=np.float32)
    w2 = np.asarray(W2, dtype=np.float32)
    b1 = np.asarray(b1, dtype=np.float32)
    b2 = np.asarray(b2, dtype=np.float32)

    # w1p[p, rc*HID + hc*128 + j] = W1[hc*128+j, rc*128+p]
    w1p = np.ascontiguousarray(
        w1.reshape(HC, 128, RC, 128).transpose(3, 2, 0, 1).reshape(128, RC * HID)
    ).astype(bf16)
    w1p = np.concatenate(
        [np.zeros((128, XT_W), dtype=bf16), w1p], axis=1
    )  # head columns hold xt, filled per core
    # w2p[p, oc*HC*512 + hc*512 + o] = W2[oc*512+o, hc*128+p]  (oc-major)
    w2p = np.ascontiguousarray(
        w2.reshape(OC, 512, HC, 128).transpose(3, 0, 2, 1).reshape(128, HC * OUT)
    ).astype(bf16)
    # aux row: b1 | b2 | ones128
    auxp = np.zeros((1, AUX_W), dtype=np.float32)
    auxp[0, 0:HID] = b1
    auxp[0, HID : HID + OUT] = b2
    auxp[0, AUX_ONES : AUX_ONES + 128] = 1.0

    in_maps = []
    for c in range(N_CORES):
        xtc = rep[c * BPC : (c + 1) * BPC].T  # [R, BPC]
        # xt[p, rc*BPC + m] = rep[m, rc*128+p] | ones row
        xtp = np.zeros((128, XT_W), dtype=np.float32)
        xtp[:, 0 : RC * BPC] = (
            xtc.reshape(RC, 128, BPC).transpose(1, 0, 2).reshape(128, RC * BPC)
        )
        xtp[0, ONE0 : ONE0 + BPC] = 1.0
        w1c = w1p.copy()
        w1c[:, 0:XT_W] = xtp.astype(bf16)
        in_maps.append(
            {
                "aux": auxp.astype(bf16),
                "w1": w1c,
                "w2": w2p,
            }
        )
    return in_maps


def run_sharded(representation, W1, b1, W2, b2, **run_kwargs):
    """Compile+run on 8 cores; returns (full_output, BassKernelResults)."""
    from concourse.bass_utils import run_bass_kernel_spmd

    nc = _get_nc()
    in_maps = _prep_in_maps(representation, W1, b1, W2, b2)
    res = run_bass_kernel_spmd(nc, in_maps, core_ids=list(range(N_CORES)), **run_kwargs)
    full = np.concatenate(
        [np.asarray(r["out"]).astype(np.float32) for r in res.results], axis=0
    )
    return full, res


def kernel(representation, size_matrix=None, W1=None, b1=None, W2=None, b2=None):
    # size_matrix only contributes its shape in the reference (ones_like);
    # its values are unused.
    full, _ = run_sharded(representation, W1, b1, W2, b2)
    return full


# revision 20
# speedup vs baseline: 1.1129x; 1.0939x over previous
"""Trainium2 Bass kernel for nn_Decoder_14894946583396 (dense_mlp).

Reference computation:
    sized = broadcast(representation[B,1,R] -> [B,S,R])   (ones @ rep)
    h     = relu(sized @ W1^T + b1)                       [B,S,HID]
    out   = h @ W2^T + b2                                 [B,S,OUT]

Every position s within batch b receives the identical input row, so
    row[b] = relu(rep[b] @ W1^T + b1) @ W2^T + b2         [B,OUT]
    out[b, s, :] = row[b]  for all s

Data-parallel across 8 NeuronCores: 4 batches per core, replicated
weights.  The 8 MiB bf16 output stream is the roofline; the schedule
minimizes head latency before the first output DMA and keeps the DMA
descriptors >= 4 KiB (small descriptors measurably cut per-engine DMA
throughput from ~25 GB/s to ~18 GB/s):

  * Inputs ride the sync HWDGE ring (xt, aux, W1 in 2 halves) --
    the highest-priority queue, drained first -- and w2's two halves
    ride the scalar HWDGE ring.  DMA queues drain in strict priority
    order (sync > scalar > SWDGE), and a DMA's completion semaphore
    fires ~1.8us after its last byte (HBM receipt round trip), so the
    weight layout is chosen to put W1's last byte as early as
    possible.
  * L1 is rc-major over 2 chunks; b1 rides mid-accumulation-group as
    a K=1 ones-matmul; relu on ACT casts to bf16.
  * L2 produces the REPLICATED output directly: for each (batch, oc)
    a 4-matmul group uses lhsT = ht[:, column b] broadcast along the
    free dim to 128 columns, so PSUM receives row[b] already
    replicated across all 128 partitions.  No selector matmul, no
    intermediate y tile, two fewer engine hops before the first
    output byte.  b2 opens each group as a K=1 start-matmul (needs
    only aux, so it runs before relu even lands).
  * One DVE copy per (batch, oc) writes the PSUM block TWICE into a
    double-row tile yt2[b] = [row | row], giving the output DMA a
    real 2048-elem contiguous source run -> 4 KiB descriptors.
  * 4 output DMAs (one per batch, 2 MiB each): b0 on sync, b1 on
    scalar (HWDGE count = 6 inputs + 2 outputs = exactly the 8
    lanes), b2/b3 on the SWDGE ring.  Each carries only its copy-tick
    wait.
  * Output stored bf16 (halves the write stream); host upcasts during
    the gather.  Total rounding ~4e-3, well under the 2e-2 gate.

A chain of 1-wait SP nops pre-observes every final tick for the
TileContext exit drain.
"""

import sys

import numpy as np

if "/opt/trn_rl_repo" not in sys.path:
    sys.path.insert(0, "/opt/trn_rl_repo")

B, S, R = 32, 1024, 1024
HID, OUT = 512, 1024
N_CORES = 8
BPC = B // N_CORES  # batches per core

RC = R // 128  # layer-1 contraction chunks
HC = HID // 128  # layer-2 contraction chunks
OC = OUT // 512  # 512-wide output column chunks

N_COPIES = S // 128  # broadcast factor per output DMA (0-stride AP)

ONE0 = RC * BPC  # ones row offset in xt
XT_W = ONE0 + BPC  # x^T | ones

# aux row 0: b1 (cols 0-511) | b2 (512-1535) | ones128 (1536-1663)
AUX_ONES = HID + OUT
AUX_W = HID + OUT + 128

_CACHED_NC = None


def _build_nc():
    import concourse.bass as bass
    import concourse.mybir as mybir
    from concourse.tile import TileContext, add_dep_helper

    f32 = mybir.dt.float32
    bf16 = mybir.dt.bfloat16
    relu = mybir.ActivationFunctionType.Relu
    nc = bass.Bass()

    aux = nc.dram_tensor("aux", [1, AUX_W], bf16, kind="ExternalInput")
    # xt (x^T | ones) rides as the head columns of the w1 tensor -- one
    # DMA, one completion lane; L1 needs both at the same time anyway
    w1 = nc.dram_tensor("w1", [128, XT_W + RC * HID], bf16, kind="ExternalInput")
    w2 = nc.dram_tensor("w2", [128, HC * OUT], bf16, kind="ExternalInput")
    out = nc.dram_tensor("out", [BPC, S, OUT], bf16, kind="ExternalOutput")

    with TileContext(nc) as tc:
        with (
            tc.tile_pool(name="const", bufs=1) as cpool,
            tc.tile_pool(name="psum_h", bufs=1, space="PSUM") as pp_h,
            tc.tile_pool(name="psum_bc", bufs=3, space="PSUM") as pp_bc,
        ):
            # ---- input DMAs ------------------------------------------------
            # EVERYTHING rides the sync ring as one deep FIFO: aux (tiny,
            # first), xt+W1 (one DMA), W2, then the five output DMAs.
            # Queues drain serially in long bursts, so a single deep queue
            # keeps all 16 SDMA engines ~99% busy with no handoff gaps,
            # and the FIFO order IS the dependency order.
            aux_sb = cpool.tile([1, AUX_W], bf16, tag="aux")
            dma_aux = nc.sync.dma_start(out=aux_sb[0:1, :], in_=aux[0:1, :])
            w1_sb = cpool.tile([128, XT_W + RC * HID], bf16, tag="w1")
            w1_dmas = [nc.sync.dma_start(out=w1_sb[:, :], in_=w1[:, :])]
            xt_sb = w1_sb  # xt = head columns of the w1 tile
            w2_sb = cpool.tile([128, HC * OUT], bf16, tag="w2")
            w2_cols = HC * OUT // OC
            w2_dmas = [
                nc.sync.dma_start(
                    out=w2_sb[:, c * w2_cols : (c + 1) * w2_cols],
                    in_=w2[:, c * w2_cols : (c + 1) * w2_cols],
                )
                for c in range(OC)
            ]
            dma_xt = w1_dmas[0]

            # ---- L1: H^T[h, m] = W1 @ x + b1, relu -------------------------
            ph = [
                pp_h.tile([128, BPC], f32, tag=f"h{hc}", name=f"ph{hc}")
                for hc in range(HC)
            ]
            # rc-major; one PSUM
            # bank per hc keeps each accumulation group sequential within
            # its bank.
            for rc in range(RC - 1):
                for hc in range(HC):
                    nc.tensor.matmul(
                        ph[hc][:, :],
                        lhsT=w1_sb[:, XT_W + rc * HID + hc * 128 : XT_W + rc * HID + (hc + 1) * 128],
                        rhs=xt_sb[:, rc * BPC : (rc + 1) * BPC],
                        start=(rc == 0),
                        stop=False,
                        skip_group_check=True,
                    )
            # b1 rides mid-group: ph[h, m] += b1[h] * 1
            for hc in range(HC):
                nc.tensor.matmul(
                    ph[hc][:, :],
                    lhsT=aux_sb[0:1, hc * 128 : (hc + 1) * 128],
                    rhs=xt_sb[0:1, ONE0 : ONE0 + BPC],
                    start=False,
                    stop=False,
                    skip_group_check=True,
                )
            rc = RC - 1
            for hc in range(HC):
                nc.tensor.matmul(
                    ph[hc][:, :],
                    lhsT=w1_sb[:, XT_W + rc * HID + hc * 128 : XT_W + rc * HID + (hc + 1) * 128],
                    rhs=xt_sb[:, rc * BPC : (rc + 1) * BPC],
                    start=False,
                    stop=True,
                    skip_group_check=True,
                )
            ht_sb = cpool.tile([128, HC * BPC], bf16, tag="ht")
            relus = []
            for hc in range(HC):
                r = nc.scalar.activation(
                    ht_sb[:, hc * BPC : (hc + 1) * BPC],
                    ph[hc][:, :],
                    relu,
                )
                relus.append(r)

            # ---- L2, replicated directly, per (batch, oc half) -------------
            yts = [
                cpool.tile([128, OUT], bf16, tag=f"yt{b}", name=f"yt{b}")
                for b in range(BPC)
            ]
            out_dmas = []
            last_dve = None
            w2_seen = [False] * OC
            for b in range(BPC):
                for oc in range(OC):
                    pb = pp_bc.tile([128, 512], f32, tag="bc", name=f"pb{b}_{oc}")
                    # b2 opens the group (needs only aux -- runs early)
                    nc.tensor.matmul(
                        pb[:, :],
                        lhsT=aux_sb[0:1, AUX_ONES : AUX_ONES + 128],
                        rhs=aux_sb[0:1, HID + oc * 512 : HID + (oc + 1) * 512],
                        start=True,
                        stop=False,
                    )
                    if not w2_seen[oc]:
                        # PE stalls here only on the w2 half it is about to
                        # consume; keeps the matmuls' wait slot free for
                        # their relu-tick wait.
                        w2_seen[oc] = True
                        wn2 = nc.tensor.nop(nofuse=True)
                        add_dep_helper(
                            wn2.ins,
                            w2_dmas[oc].ins,
                            sync=True,
                            reason=f"observe w2-{oc}",
                        )
                    for hc in range(HC):
                        # lhsT = ht column for batch b, broadcast along the
                        # free dim -> PSUM gets row[b] replicated across all
                        # 128 partitions.  No selector matmul needed.
                        nc.tensor.matmul(
                            pb[:, :],
                            lhsT=ht_sb[
                                :, hc * BPC + b : hc * BPC + b + 1
                            ].to_broadcast((128, 128)),
                            rhs=w2_sb[
                                :,
                                oc * HC * 512
                                + hc * 512 : oc * HC * 512
                                + (hc + 1) * 512,
                            ],
                            start=False,
                            stop=(hc == HC - 1),
                        )
                    # plain full-tile read: the PSUM dep tracker sees the
                    # copy as covering the writer, so the next group's
                    # start-matmul carries only this copy-tick wait
                    last_dve = nc.vector.tensor_copy(
                        yts[b][:, oc * 512 : (oc + 1) * 512], pb[:, :]
                    )
                    if b == 0:
                        # batch 0 ships per oc half (1 MiB) the moment its
                        # copy lands -- trims ~2us off the first output's
                        # start at a small descriptor-size cost
                        d = nc.sync.dma_start(
                            out=out[0, :, oc * 512 : (oc + 1) * 512].rearrange(
                                "(p c) o -> p c o", c=N_COPIES
                            ),
                            in_=yts[0][:, oc * 512 : (oc + 1) * 512]
                            .rearrange("p (c o) -> p c o", c=1)
                            .to_broadcast((128, N_COPIES, 512)),
                        )
                        out_dmas.append(d)
                if b == 0:
                    continue
                # one 2 MiB DMA per later batch writes all S rows via a
                # 0-stride broadcast source AP (2 KiB contiguous runs on
                # the DRAM side); both oc copies are DVE so the DMA
                # carries one wait
                d = (nc.gpsimd if b == BPC - 1 else nc.sync).dma_start(
                    out=out[b, :, :].rearrange("(p c) o -> p c o", c=N_COPIES),
                    in_=yts[b][:, :]
                    .rearrange("p (c o) -> p c o", c=1)
                    .to_broadcast((128, N_COPIES, OUT)),
                )
                out_dmas.append(d)

            # single-sync-wait discipline for the TileContext exit drain:
            # chain SP nops, one dependency each, so SP's vector clock
            # observes every DMA lane / engine tick before the drain.
            tail = (
                out_dmas
                + w1_dmas
                + w2_dmas
                + [dma_xt, dma_aux, relus[-1], last_dve]
            )
            tail = [t for t in tail if t is not None]
            for d in tail:
                tn = nc.sync.nop(nofuse=True)
                add_dep_helper(
                    tn.ins, d.ins, sync=True, reason="observe final ticks pre-drain"
                )

    return nc


def _get_nc():
    global _CACHED_NC
    if _CACHED_NC is None:
        _CACHED_NC = _build_nc()
    return _CACHED_NC


def _prep_in_maps(representation, W1, b1, W2, b2):
    import ml_dtypes

    bf16 = ml_dtypes.bfloat16

    rep = np.asarray(representation, dtype=np.float32).reshape(B, R)
    w1 = np.asarray(W1, dtype=np.float32)
    w2 = np.asarray(W2, dtype=np.float32)
    b1 = np.asarray(b1, dtype=np.float32)
    b2 = np.asarray(b2, dtype=np.float32)

    # w1p[p, rc*HID + hc*128 + j] = W1[hc*128+j, rc*128+p]
    w1p = np.ascontiguousarray(
        w1.reshape(HC, 128, RC, 128).transpose(3, 2, 0, 1).reshape(128, RC * HID)
    ).astype(bf16)
    w1p = np.concatenate(
        [np.zeros((128, XT_W), dtype=bf16), w1p], axis=1
    )  # head columns hold xt, filled per core
    # w2p[p, oc*HC*512 + hc*512 + o] = W2[oc*512+o, hc*128+p]  (oc-major)
    w2p = np.ascontiguousarray(
        w2.reshape(OC, 512, HC, 128).transpose(3, 0, 2, 1).reshape(128, HC * OUT)
    ).astype(bf16)
    # aux row: b1 | b2 | ones128
    auxp = np.zeros((1, AUX_W), dtype=np.float32)
    auxp[0, 0:HID] = b1
    auxp[0, HID : HID + OUT] = b2
    auxp[0, AUX_ONES : AUX_ONES + 128] = 1.0

    in_maps = []
    for c in range(N_CORES):
        xtc = rep[c * BPC : (c + 1) * BPC].T  # [R, BPC]
        # xt[p, rc*BPC + m] = rep[m, rc*128+p] | ones row
        xtp = np.zeros((128, XT_W), dtype=np.float32)
        xtp[:, 0 : RC * BPC] = (
            xtc.reshape(RC, 128, BPC).transpose(1, 0, 2).reshape(128, RC * BPC)
        )
        xtp[0, ONE0 : ONE0 + BPC] = 1.0
        w1c = w1p.copy()
        w1c[:, 0:XT_W] = xtp.astype(bf16)
        in_maps.append(
            {
                "aux": auxp.astype(bf16),
                "w1": w1c,
                "w2": w2p,
            }
        )
    return in_maps


def run_sharded(representation, W1, b1, W2, b2, **run_kwargs):
    """Compile+run on 8 cores; returns (full_output, BassKernelResults)."""
    from concourse.bass_utils import run_bass_kernel_spmd

    nc = _get_nc()
    in_maps = _prep_in_maps(representation, W1, b1, W2, b2)
    res = run_bass_kernel_spmd(nc, in_maps, core_ids=list(range(N_CORES)), **run_kwargs)
    full = np.concatenate(
        [np.asarray(r["out"]).astype(np.float32) for r in res.results], axis=0
    )
    return full, res


def kernel(representation, size_matrix=None, W1=None, b1=None, W2=None, b2=None):
    # size_matrix only contributes its shape in the reference (ones_like);
    # its values are unused.
    full, _ = run_sharded(representation, W1, b1, W2, b2)
    return full
